# revision 1
# baseline (speedup 1.0000x reference)
"""Trainium2 Bass kernel for nn_PluckettLuceKeibaBetting.

B=8192 races x H=18 horses -> (8192, 6360) bet-type probabilities.
Pure data-parallel across 8 NeuronCores (1024 races each, 8 tiles of 128).

Every segment is computed in closed form from the Plackett-Luce factorization
p(f,s,t) = (ef/d1)(es/(d1-ef))(et/(d1-ef-es)) rather than scatter-adding 4896
permutation probabilities:

  tansho[j]    = ej/d1
  umatan[f,s]  = exp(sf+ss) * r1 * g_f            g_j = 1/(d1-ej)
  umaren{a,b}  = exp(sa+sb) * r1 * (ga+gb)
  q[f,s]       = umatan * h306                    h306 = 1/(d1-ef-es)
  UQ{a,b}      = umaren * h153
  wide{a,b}    = umaren - (ea+eb)*UQ + [ (ea+eb)(rqa+rqb) - (ea*rqa+eb*rqb) ]
                 with rq_j = sum of UQ over pairs containing j, w_j = ej*rq_j
  fukusho      = tansho + P2nd (+P3rd if >7 horses running)
  sanrenpuku   = exp(sa+sb+sc) * r1 * BR,  BR = 3-pair gather of h*(gx+gy)
  sanrentan    = q[opair] * e[third]  (broadcast multiply; perm order is
                 ordered-pair-major with 16 consecutive thirds)

All gathers are constant 0/1 matrices applied via TensorE matmuls. Data
gathers (es, g, uq, hpg, rq, w) run in float32r (e8m11, full fp32 rate);
score-sum gathers that feed exp() stay exact float32. ScalarE uses only the
exp_and_others table set (Exp/Copy) -> a single ACT_TABLE_LOAD. Reciprocals
use the custom-DVE reciprocal_approx_fast.
"""

import itertools
import numpy as np

H = 18
B = 8192
NCORES = 8
BC = B // NCORES  # 1024 races per core
P = 128
NT = BC // P      # 8 race-tiles per core
N_PAIR = 153
N_OPAIR = 306
N_TRIP = 816
N_PERM = 4896
OUT_D = 6360

OFF_TANSHO = 0
OFF_FUKU = 18
OFF_UMAREN = 36
OFF_WIDE = 189
OFF_UMATAN = 342
OFF_SANPUKU = 648
OFF_SANTAN = 1464

ET_CHUNKS = [(i * 512, 512) for i in range(9)] + [(9 * 512, 288)]
# chunk indices whose q*ET multiply runs on GPSIMD (via an ACT PSUM->SBUF copy)
GP_CHUNKS = frozenset({6, 7, 8, 9})

# TRN (transposed per-horse tile) row layout: ST 0:18, EST 32:50, GT 64:82
R_S, R_ES, R_G = 0, 32, 64
TRNROWS = 96


def _build_consts():
    perms3 = np.array(list(itertools.permutations(range(H), 3)), dtype=np.int32)
    T = perms3[:, 2]
    opairs = list(itertools.permutations(range(H), 2))
    combos2 = list(itertools.combinations(range(H), 2))
    combos3 = list(itertools.combinations(range(H), 3))

    pair_id = {}
    for i, (a, b) in enumerate(combos2):
        pair_id[(a, b)] = i
        pair_id[(b, a)] = i

    M_2HOT = np.zeros((18, N_PAIR), np.float32)
    for j, (a, b) in enumerate(combos2):
        M_2HOT[a, j] += 1.0
        M_2HOT[b, j] += 1.0

    # ---- float32r data gathers (lhsT = TRN[0:96] or transposed data) ----
    # merged pair gathers: C_M1 -> [GF(306) | SE153(153)], C_M2 -> [SE306(306) | SG(153)]
    C_M1 = np.zeros((TRNROWS, 460), np.float32)
    C_M2 = np.zeros((TRNROWS, 460), np.float32)
    for j, (f, s) in enumerate(opairs):
        C_M1[R_G + f, j] += 1.0          # GF = g_first
        C_M2[R_ES + f, j] += 1.0         # SE306 = ef+es
        C_M2[R_ES + s, j] += 1.0
    C_M1[R_ES:R_ES + 18, 306:459] = M_2HOT  # SE153
    C_M2[R_G:R_G + 18, 306:459] = M_2HOT    # SG

    G_T = np.zeros((TRNROWS, N_PERM), np.float32)       # e_third (ES rows)
    for j, t in enumerate(T):
        G_T[R_ES + t, j] = 1.0

    M_BR = np.zeros((N_PAIR, N_TRIP), np.float32)
    for j, (a, b, c) in enumerate(combos3):
        M_BR[pair_id[(a, b)], j] += 1.0
        M_BR[pair_id[(a, c)], j] += 1.0
        M_BR[pair_id[(b, c)], j] += 1.0

    M_RQ = np.zeros((N_PAIR, 18), np.float32)
    for i, (a, b) in enumerate(combos2):
        M_RQ[i, a] += 1.0
        M_RQ[i, b] += 1.0

    # ---- exact float32 score-sum gathers (lhsT = trnx (18,128)) ----
    C_L2ORD = np.zeros((18, N_OPAIR), np.float32)
    for j, (f, s) in enumerate(opairs):
        C_L2ORD[f, j] += 1.0
        C_L2ORD[s, j] += 1.0
    C_SC2 = M_2HOT.copy()
    C_L3 = np.zeros((18, N_TRIP), np.float32)
    for j, (a, b, c) in enumerate(combos3):
        C_L3[a, j] += 1.0
        C_L3[b, j] += 1.0
        C_L3[c, j] += 1.0

    # stacked [uq(153) | hpg(153) | rq(18) | w(18)] transposed in chunks:
    #   A = cols 0:128 (uq 0:128), B = cols 128:256
    #   (uq 128:153 at rows 0:25, hpg 0:103 at rows 25:128), C = cols 256:342
    #   (hpg 103:153 at rows 0:50, rq at 50:68, w at 68:86)
    C_RQ_A = M_RQ[:128]
    C_RQ_B = np.zeros((128, 18), np.float32)
    C_RQ_B[0:25] = M_RQ[128:153]
    C_BR_B = np.zeros((128, N_TRIP), np.float32)
    C_BR_B[25:128] = M_BR[0:103]
    C_BR_C = np.zeros((86, N_TRIP), np.float32)
    C_BR_C[0:50] = M_BR[103:153]
    C_RQW_C = np.zeros((86, N_OPAIR), np.float32)
    C_RQW_C[50:68, 0:153] = M_2HOT   # SRQ = rqa+rqb
    C_RQW_C[68:86, 153:306] = M_2HOT  # SW2 = wa+wb
    consts_r = dict(
        C_M1=C_M1, C_M2=C_M2, G_T=G_T,
        C_BR_B=C_BR_B, C_BR_C=C_BR_C,
        C_RQ_A=C_RQ_A, C_RQ_B=C_RQ_B, C_RQW_C=C_RQW_C,
    )
    C_L2SC = np.concatenate([C_L2ORD, C_SC2], axis=1)  # (18, 459)
    consts_x = dict(
        C_L2SC=C_L2SC, C_L3=C_L3,
        IDENT=np.eye(128, dtype=np.float32),
    )
    return consts_r, consts_x


def _build_body(ctx, tc, out_ap, scores_ap, maskneg_ap, consts_r, consts_x):
    import concourse.mybir as mybir

    nc = tc.nc
    f32 = mybir.dt.float32
    f32r = mybir.dt.float32r
    Exp = mybir.ActivationFunctionType.Exp
    MUL = mybir.AluOpType.mult
    SUB = mybir.AluOpType.subtract
    ADD = mybir.AluOpType.add

    def r(ap):
        return ap.bitcast(f32r)

    def mmr(out, lhsT, rhs, **kw):  # float32r full-rate matmul
        nc.tensor.matmul(out, r(lhsT), r(rhs), **kw)

    # ---- persistent constants ----
    cpool = ctx.enter_context(tc.tile_pool(name="consts", bufs=1))
    C = {}
    for name, arr in consts_r.items():
        dram = nc.inline_tensor(arr, name=f"c_{name}")
        t = cpool.tile(list(arr.shape), f32, tag=f"c_{name}")
        nc.sync.dma_start(out=r(t[:]), in_=r(dram.ap()))
        C[name] = t
    for name, arr in consts_x.items():
        dram = nc.inline_tensor(arr, name=f"c_{name}")
        t = cpool.tile(list(arr.shape), f32, tag=f"c_{name}")
        nc.sync.dma_start(out=t[:], in_=dram.ap())
        C[name] = t

    inpool = ctx.enter_context(tc.tile_pool(name="inp", bufs=1))
    mk = inpool.tile([P, NT], f32, tag="maskneg")
    nc.sync.dma_start(out=mk[:], in_=maskneg_ap.rearrange("(n p) o -> p (n o)", p=P))

    outp = ctx.enter_context(tc.tile_pool(name="out", bufs=2))
    wk = ctx.enter_context(tc.tile_pool(name="work", bufs=2))
    pps = ctx.enter_context(tc.tile_pool(name="ppsmall", bufs=4, space="PSUM"))
    ppb = ctx.enter_context(tc.tile_pool(name="ppbig", bufs=1, space="PSUM"))
    ppe = ctx.enter_context(tc.tile_pool(name="ppet", bufs=2, space="PSUM"))

    for t in range(NT):
        rows = slice(t * P, (t + 1) * P)
        ot = outp.tile([P, OUT_D], f32, tag="ot")
        mask_t = mk[:, t:t + 1]

        # ---- per-horse (race-major) ----
        stk = wk.tile([P, TRNROWS], f32, tag="stk")  # cols: S|pad|ES|pad|G|pad
        nc.gpsimd.memset(stk[:], 0.0)
        S = stk[:, R_S:R_S + 18]
        ES = stk[:, R_ES:R_ES + 18]
        G = stk[:, R_G:R_G + 18]
        nc.sync.dma_start(out=S, in_=scores_ap[rows])

        d1 = wk.tile([P, 1], f32, tag="d1")
        nc.scalar.activation(ES, S, Exp, accum_out=d1[:])
        r1 = wk.tile([P, 1], f32, tag="r1")
        nc.vector.reciprocal(r1[:], d1[:])
        dmg = wk.tile([P, 18], f32, tag="dmg")  # d1 - e
        nc.vector.tensor_scalar(dmg[:], ES, d1[:], -1.0, op0=SUB, op1=MUL)
        nc.vector.reciprocal_approx_fast(out=G, in_=dmg[:])

        # tansho
        nc.vector.tensor_scalar_mul(ot[:, 0:18], ES, r1[:])
        # fuku2 = tansho + e*(SS - z), z = e*g/d1
        z = wk.tile([P, 18], f32, tag="z")
        SSc = wk.tile([P, 1], f32, tag="SSc")
        nc.vector.scalar_tensor_tensor(
            z[:], in0=ES, scalar=r1[:], in1=G, op0=MUL, op1=MUL, accum_out=SSc[:])
        np2 = wk.tile([P, 18], f32, tag="np2")  # (z-SS)*e = -P2nd
        nc.vector.scalar_tensor_tensor(
            np2[:], in0=z[:], scalar=SSc[:], in1=ES, op0=SUB, op1=MUL)
        fuku2 = wk.tile([P, 18], f32, tag="fuku2")
        nc.gpsimd.tensor_sub(fuku2[:], ot[:, 0:18], np2[:])

        # ---- transpose 1 (exact f32): stk -> (96,128) ----
        ps_trn = pps.tile([TRNROWS, P], f32, tag="ps")
        nc.tensor.matmul(ps_trn[:], stk[:], C["IDENT"][:], is_transpose=True)
        trn = wk.tile([TRNROWS, P], f32, tag="trn")      # f32r-rounded copy
        nc.scalar.copy(r(trn[:]), ps_trn[:])
        trnx = wk.tile([18, P], f32, tag="trnx")         # exact scores^T
        nc.scalar.copy(trnx[:], ps_trn[R_S:R_S + 18])

        # ---- gathers (merged) ----
        ps_m1 = pps.tile([P, 460], f32, tag="ps")  # [GF | SE153]
        mmr(ps_m1[:], trn[:], C["C_M1"][:], start=True, stop=True)
        ps_m2 = pps.tile([P, 460], f32, tag="ps")  # [SE306 | SG]
        mmr(ps_m2[:], trn[:], C["C_M2"][:], start=True, stop=True)
        ps_gf = ps_m1[:, 0:306]
        SE153 = ps_m1[:, 306:459]
        ps_se306 = ps_m2[:, 0:306]
        SG = ps_m2[:, 306:459]

        ps_l2sc = pps.tile([P, 459], f32, tag="ps")  # [L2ord | SC2]
        nc.tensor.matmul(ps_l2sc[:], trnx[:], C["C_L2SC"][:], start=True, stop=True)
        ps_l2o = ps_l2sc[:, 0:306]
        ps_sc2 = ps_l2sc[:, 306:459]

        # umatan = exp(L2ord)*r1*GF
        e2o = wk.tile([P, N_OPAIR], f32, tag="e2o")
        nc.scalar.activation(e2o[:], ps_l2o, Exp)
        nc.vector.scalar_tensor_tensor(
            ot[:, OFF_UMATAN:OFF_UMATAN + N_OPAIR],
            in0=e2o[:], scalar=r1[:], in1=ps_gf, op0=MUL, op1=MUL)

        # q = umatan * h306
        h306 = wk.tile([P, N_OPAIR], f32, tag="h306")
        nc.vector.tensor_scalar(h306[:], ps_se306, d1[:], -1.0, op0=SUB, op1=MUL)
        nc.vector.reciprocal_approx_fast(out=h306[:], in_=h306[:])
        q = wk.tile([P, N_OPAIR], f32, tag="q")
        nc.gpsimd.tensor_mul(q[:], ot[:, OFF_UMATAN:OFF_UMATAN + N_OPAIR], h306[:])

        # umaren = exp(SC2)*r1*SG ; h153 ; hpg ; UQ
        esc2 = wk.tile([P, N_PAIR], f32, tag="esc2")
        nc.scalar.activation(esc2[:], ps_sc2, Exp)
        nc.vector.scalar_tensor_tensor(
            ot[:, OFF_UMAREN:OFF_UMAREN + N_PAIR],
            in0=esc2[:], scalar=r1[:], in1=SG, op0=MUL, op1=MUL)
        d3p = wk.tile([P, N_PAIR], f32, tag="d3p")
        nc.vector.tensor_scalar(d3p[:], SE153, d1[:], -1.0, op0=SUB, op1=MUL)
        h153 = wk.tile([P, N_PAIR], f32, tag="h153")
        nc.vector.reciprocal_approx_fast(out=h153[:], in_=d3p[:])
        # stacked [uq(153) | hpg(153) | rq(18) | w(18)] for merged transposes
        stk2 = wk.tile([P, 342], f32, tag="stk2")
        uq = stk2[:, 0:153]
        hpg = stk2[:, 153:306]
        nc.vector.tensor_mul(hpg, h153[:], SG)
        nc.gpsimd.tensor_mul(uq, ot[:, OFF_UMAREN:OFF_UMAREN + N_PAIR], h153[:])

        # ---- transpose 2 (f32) + f32r-rounding copies (chunks A, B) ----
        ps_ta = pps.tile([P, P], f32, tag="ps")
        nc.tensor.matmul(ps_ta[:], stk2[:, 0:128], C["IDENT"][:], is_transpose=True)
        ps_tb = pps.tile([P, P], f32, tag="ps")
        nc.tensor.matmul(ps_tb[:], stk2[:, 128:256], C["IDENT"][:], is_transpose=True)
        ta = wk.tile([P, P], f32, tag="ta")
        nc.scalar.copy(r(ta[:]), ps_ta[:])
        tb = wk.tile([P, P], f32, tag="tb")
        nc.scalar.copy(r(tb[:]), ps_tb[:])

        # rq
        ps_rq = pps.tile([P, 18], f32, tag="ps")
        mmr(ps_rq[:], ta[:], C["C_RQ_A"][:], start=True, stop=False)
        mmr(ps_rq[:], tb[:], C["C_RQ_B"][:], start=False, stop=True)

        # fukusho
        rqs = wk.tile([P, 1], f32, tag="rqs")
        nc.vector.tensor_reduce(rqs[:], ps_rq[:], axis=mybir.AxisListType.X, op=ADD)
        sq = wk.tile([P, 1], f32, tag="sq")
        nc.vector.tensor_scalar_mul(sq[:], rqs[:], 0.5)
        np3 = wk.tile([P, 18], f32, tag="np3")
        nc.vector.scalar_tensor_tensor(
            np3[:], in0=ps_rq[:], scalar=sq[:], in1=ES, op0=SUB, op1=MUL)
        nc.vector.scalar_tensor_tensor(
            ot[:, OFF_FUKU:OFF_FUKU + 18], in0=np3[:], scalar=mask_t,
            in1=fuku2[:], op0=MUL, op1=ADD)

        # ---- wide: rq/w into stk2 cols 306:342, transpose chunk C ----
        nc.vector.tensor_copy(out=stk2[:, 306:324], in_=ps_rq[:])
        nc.vector.tensor_mul(stk2[:, 324:342], ES, ps_rq[:])
        ps_tc = pps.tile([86, P], f32, tag="ps")
        nc.tensor.matmul(ps_tc[:], stk2[:, 256:342], C["IDENT"][:], is_transpose=True)
        tc_t = wk.tile([86, P], f32, tag="tc_t")
        nc.scalar.copy(r(tc_t[:]), ps_tc[:])
        ps_srqw = pps.tile([P, N_OPAIR], f32, tag="ps")  # [SRQ | SW2]
        mmr(ps_srqw[:], tc_t[:], C["C_RQW_C"][:], start=True, stop=True)

        n1 = wk.tile([P, N_PAIR], f32, tag="n1")  # -(SE153*SRQ)
        nc.vector.scalar_tensor_tensor(
            n1[:], in0=d3p[:], scalar=d1[:], in1=ps_srqw[:, 0:153],
            op0=SUB, op1=MUL)
        t2 = wk.tile([P, N_PAIR], f32, tag="t2")  # n1 + SW2 = -cross
        nc.vector.tensor_add(t2[:], n1[:], ps_srqw[:, 153:306])
        nm1 = wk.tile([P, N_PAIR], f32, tag="nm1")  # -(SE153*UQ)
        nc.vector.scalar_tensor_tensor(
            nm1[:], in0=d3p[:], scalar=d1[:], in1=uq, op0=SUB, op1=MUL)
        g1 = wk.tile([P, N_PAIR], f32, tag="g1")
        nc.gpsimd.tensor_add(g1[:], ot[:, OFF_UMAREN:OFF_UMAREN + N_PAIR], nm1[:])
        nc.gpsimd.tensor_sub(ot[:, OFF_WIDE:OFF_WIDE + N_PAIR], g1[:], t2[:])

        # ---- sanrenpuku ----
        ps_l3 = ppb.tile([P, N_TRIP], f32, tag="big")
        nc.tensor.matmul(ps_l3[:, 0:512], trnx[:], C["C_L3"][:, 0:512],
                         start=True, stop=True)
        nc.tensor.matmul(ps_l3[:, 512:816], trnx[:], C["C_L3"][:, 512:816],
                         start=True, stop=True)
        eabc = wk.tile([P, N_TRIP], f32, tag="eabc")
        nc.scalar.activation(eabc[:], ps_l3[:], Exp)

        ps_br = ppb.tile([P, N_TRIP], f32, tag="big")
        for lo, hi in ((0, 512), (512, 816)):
            mmr(ps_br[:, lo:hi], tb[:], C["C_BR_B"][:, lo:hi], start=True, stop=False)
            mmr(ps_br[:, lo:hi], tc_t[:], C["C_BR_C"][:, lo:hi], start=False, stop=True)
        nc.vector.scalar_tensor_tensor(
            ot[:, OFF_SANPUKU:OFF_SANPUKU + N_TRIP],
            in0=eabc[:], scalar=r1[:], in1=ps_br[:], op0=MUL, op1=MUL)

        # ---- sanrentan: P = ET * q (broadcast over 16 thirds) ----
        for ci, (c0, w) in enumerate(ET_CHUNKS):
            ps_et = ppe.tile([P, 512], f32, tag="et")
            mmr(ps_et[:, 0:w], trn[:], C["G_T"][:, c0:c0 + w], start=True, stop=True)
            npair = w // 16
            qb = q[:, c0 // 16: c0 // 16 + npair].unsqueeze(2).broadcast_to(
                [P, npair, 16])
            dst = ot[:, OFF_SANTAN + c0: OFF_SANTAN + c0 + w].rearrange(
                "p (a b) -> p a b", b=16)
            if ci in GP_CHUNKS:
                et_sb = wk.tile([P, 512], f32, tag="et_sb")
                nc.scalar.copy(et_sb[:, 0:w], ps_et[:, 0:w])
                src = et_sb[:, 0:w].rearrange("p (a b) -> p a b", b=16)
                nc.gpsimd.tensor_tensor(out=dst, in0=src, in1=qb, op=MUL)
            else:
                src = ps_et[:, 0:w].rearrange("p (a b) -> p a b", b=16)
                nc.vector.tensor_tensor(out=dst, in0=src, in1=qb, op=MUL)

        nc.sync.dma_start(out=out_ap[rows], in_=ot[:])


def _build_bass():
    from contextlib import ExitStack
    import concourse.bacc as bacc
    import concourse.mybir as mybir
    import concourse.tile as tile

    consts_r, consts_x = _build_consts()
    nc = bacc.Bacc("TRN2", target_bir_lowering=False, debug=False,
                   enable_asserts=False, num_devices=NCORES)
    f32 = mybir.dt.float32
    scores = nc.dram_tensor("scores", (BC, H), f32, kind="ExternalInput").ap()
    maskneg = nc.dram_tensor("maskneg", (BC, 1), f32, kind="ExternalInput").ap()
    out = nc.dram_tensor("out", (BC, OUT_D), f32, kind="ExternalOutput").ap()

    with tile.TileContext(nc) as tc:
        with ExitStack() as ctx:
            _build_body(ctx, tc, out, scores, maskneg, consts_r, consts_x)
    nc.compile()
    return nc


_cached_nc = None


def _get_nc():
    global _cached_nc
    if _cached_nc is None:
        _cached_nc = _build_bass()
    return _cached_nc


def kernel(scores, num_horses_running, _trace=False, _tmpdir=None):
    from concourse.bass_utils import run_bass_kernel_spmd

    nc = _get_nc()
    scores = np.ascontiguousarray(np.asarray(scores), dtype=np.float32)
    nhr = np.asarray(num_horses_running)
    maskneg = np.where(nhr > 7, -1.0, 0.0).astype(np.float32).reshape(B, 1)

    in_maps = [
        {"scores": scores[c * BC:(c + 1) * BC],
         "maskneg": np.ascontiguousarray(maskneg[c * BC:(c + 1) * BC])}
        for c in range(NCORES)
    ]
    res = run_bass_kernel_spmd(nc, in_maps, core_ids=list(range(NCORES)),
                               trace=_trace, tmpdir=_tmpdir)
    out = np.concatenate([r["out"] for r in res.results], axis=0)
    if _trace:
        return out, res
    return out

